# revision 1
# baseline (speedup 1.0000x reference)
"""CoGOL ordinal-logistic loss on 8 Trainium2 NeuronCores.

Math (per sample, target t in [1,64], logits x[0..62], cum=[0|x]):
  loss_i = sum_{j<=t-3} log_sigmoid(-x_j) + sum_{t-1<=j<=61} log_sigmoid(x_j)
           + [t>=2]*log_sigmoid(0)            (col 0 of cum; x_62 never used)
With s = clip(t-2-j, -1, 1):  the two masked sums equal
  -[ sum_{j=0}^{61} softplus(s_j * x_j) - ln2 * [2<=t<=63] ]
so with N64 = count(t==64) per core:
  loss_core = -sum softplus(s*x) - ln2 * N64
and the final result is -loss/B + a/2*sum(w^2) + b/2*sum(d[1:]^2).

Sharding: batch split 8 ways (65536 rows/core); weights flat-split 8 ways;
deltas[1:] to core 0 only (others get zeros). Each core emits one partial
scalar; host sums the 8 partials.
"""

import sys

sys.path.insert(0, "/opt/trn_rl_repo")

import numpy as np

ALPHA = 0.01
BETA = 0.05
B = 524288
KM1 = 63
NCORES = 8
BC = B // NCORES            # 65536 rows per core
R = 32                      # rows per partition per tile
ROWS_PER_TILE = 128 * R     # 2048
NTILES = BC // ROWS_PER_TILE  # 32
WPER = (3 * 512 * 512) // NCORES  # 98304 weights elements per core
LN2 = 0.6931471805599453

_PROG = None


def _build():
    import concourse.bacc as bacc
    import concourse.tile as tile
    from concourse import mybir

    # Exp and Ln both live in the "natural_log_exp_and_others" ACT table set,
    # but the table-load inserter picks the first set containing each func,
    # which ping-pongs between two sets (a ~1.3us reload per activation).
    # Blank every other set (order preserved, so set ids stay valid) to force
    # a single resident table.
    import concourse.hw_specs as hw_specs
    if not getattr(bacc, "_act_tables_pinned", False):
        _orig_get = hw_specs.get_activation_tables

        def _pinned(arch, _orig=_orig_get):
            tabs = _orig(arch)
            keep = "natural_log_exp_and_others"
            return {k: (v if k == keep else set()) for k, v in tabs.items()}

        bacc.get_activation_tables = _pinned
        bacc._act_tables_pinned = True

    f32 = mybir.dt.float32
    i32 = mybir.dt.int32
    Alu = mybir.AluOpType
    Act = mybir.ActivationFunctionType

    nc = bacc.Bacc("TRN2", target_bir_lowering=False, debug=False, num_devices=NCORES)

    logits = nc.dram_tensor("logits", [BC, KM1], f32, kind="ExternalInput")
    targets = nc.dram_tensor("targets", [BC], f32, kind="ExternalInput")
    wts = nc.dram_tensor("wts", [WPER], f32, kind="ExternalInput")
    dls = nc.dram_tensor("dls", [192], f32, kind="ExternalInput")
    out = nc.dram_tensor("out", [1, 1], f32, kind="ExternalOutput")

    with tile.TileContext(nc) as tc:
        with (
            tc.tile_pool(name="const", bufs=1) as cpool,
            tc.tile_pool(name="x", bufs=4) as xpool,
            tc.tile_pool(name="t", bufs=4) as tpool,
            tc.tile_pool(name="w", bufs=3) as wpool,
            tc.tile_pool(name="sp", bufs=3) as spool,
            tc.tile_pool(name="fin", bufs=1) as fpool,
            tc.tile_pool(name="ps", bufs=1, space="PSUM") as ppool,
        ):
            # constant: iota value j+2 for one 63-col block; the tensor_tensor
            # operand repeats it per row-block via a 0-step AP dim
            iota_i = cpool.tile([128, KM1], i32)
            nc.gpsimd.iota(iota_i[:], pattern=[[1, KM1]], base=2,
                           channel_multiplier=0)
            iota_f = cpool.tile([128, KM1], f32)
            nc.vector.tensor_copy(iota_f[:], iota_i[:])

            ones = cpool.tile([128, 1], f32)
            nc.vector.memset(ones[:], 1.0)

            # all targets up-front in per-tile layout: T[p, roff+m] =
            # targets[128*roff... tile row block] so each tile's slice is a
            # per-partition [128, r] view. One strided DMA.
            RTOT = BC // 128                      # 512 rows per partition
            tload = cpool.tile([128, RTOT], f32)
            sizes = [8, 8, 8, 8] + [R] * ((RTOT - 64) // R) + [8, 8, 8, 8]
            assert sum(sizes) == RTOT
            offs = [sum(sizes[:i]) for i in range(len(sizes))]
            nc.sync.dma_start(
                tload[:], targets.ap().rearrange("(p r) -> p r", p=128)
            )

            acc = cpool.tile([128, len(sizes)], f32)

            for k, (r, roff) in enumerate(zip(sizes, offs)):
                rows0 = roff * 128
                xt = xpool.tile([128, R, KM1], f32, tag="x")
                nc.sync.dma_start(
                    xt[:, :r, :],
                    logits.ap()[rows0:rows0 + r * 128, :]
                    .rearrange("(p r) c -> p r c", p=128),
                )
                tt = tpool.tile([128, R], f32, tag="t")
                nc.sync.dma_start(
                    tt[:, :r],
                    targets.ap()[rows0:rows0 + r * 128]
                    .rearrange("(p r) -> p r", p=128),
                )

                # w = t - (j+2);  s = clip(w, -1, 1);  arg = s * x
                wt = wpool.tile([128, R, KM1], f32, tag="w")
                nc.vector.tensor_tensor(
                    wt[:, :r, :], tt[:, :r, None].to_broadcast([128, r, KM1]),
                    iota_f[:][:, None, :].to_broadcast([128, r, KM1]),
                    Alu.subtract,
                )
                nc.vector.tensor_scalar(
                    wt[:, :r, :], wt[:, :r, :], -1.0, 1.0, Alu.max, Alu.min,
                )
                arg = wpool.tile([128, R, KM1], f32, tag="arg")
                nc.vector.tensor_tensor(
                    arg[:, :r, :], wt[:, :r, :], xt[:, :r, :], Alu.mult)

                # softplus(a) = ln(exp(a) + 1) over the first 62 columns only;
                # exp and ln share one ACT table set, and the "+1" rides the
                # activation's free bias. Row-sums accumulate into acc[:, k].
                et = spool.tile([128, R, KM1 - 1], f32, tag="et")
                nc.scalar.activation(
                    et[:, :r, :], arg[:, :r, 0:KM1 - 1], Act.Exp)
                spo = spool.tile([128, R, KM1 - 1], f32, tag="spo")
                nc.scalar.activation(
                    spo[:, :r, :], et[:, :r, :], Act.Ln, bias=1.0,
                    accum_out=acc[:, k:k + 1],
                )

            # N64 per partition: sum of max(t-63, 0) over all targets
            n64scr = tpool.tile([128, RTOT], f32, tag="tall_scr")
            n64 = fpool.tile([128, 1], f32, tag="n64")
            nc.vector.tensor_scalar(
                n64scr[:], tload[:], 63.0, 0.0, Alu.subtract, Alu.max,
                accum_out=n64[:],
            )

            # weights shard sum of squares
            wtile = wpool.tile([128, WPER // 128], f32, tag="wts")
            nc.sync.dma_start(wtile[:], wts.ap().rearrange("(p r) -> p r", p=128))
            wscr = wpool.tile([128, WPER // 128], f32, tag="wts_scr")
            wacc = fpool.tile([128, 1], f32, tag="wacc")
            nc.vector.scalar_tensor_tensor(
                wscr[:], wtile[:], 0.0, wtile[:], Alu.add, Alu.mult,
                accum_out=wacc[:],
            )

            # deltas (row 0 already dropped host-side; zeros on cores 1-7)
            dtile = fpool.tile([1, 192], f32, tag="dt")
            nc.sync.dma_start(dtile[:], dls.ap().rearrange("(p r) -> p r", p=1))
            dscr = fpool.tile([1, 192], f32, tag="dscr")
            dacc = fpool.tile([1, 1], f32, tag="dacc")
            nc.vector.scalar_tensor_tensor(
                dscr[:], dtile[:], 0.0, dtile[:], Alu.add, Alu.mult,
                accum_out=dacc[:],
            )

            # per-partition combine:
            #   comb = accP/B + n64*ln2/B + wacc*alpha/2
            accP = fpool.tile([128, 1], f32, tag="accP")
            nc.vector.reduce_sum(accP[:], acc[:], axis=mybir.AxisListType.X)
            comb = fpool.tile([128, 1], f32, tag="comb")
            nc.vector.tensor_scalar_mul(comb[:], accP[:], 1.0 / B)
            nc.vector.scalar_tensor_tensor(
                comb[:], n64[:], LN2 / B, comb[:], Alu.mult, Alu.add,
            )
            nc.vector.scalar_tensor_tensor(
                comb[:], wacc[:], ALPHA / 2.0, comb[:], Alu.mult, Alu.add,
            )

            # cross-partition sum via matmul with ones, then add delta term
            psum = ppool.tile([1, 1], f32)
            nc.tensor.matmul(psum[:], comb[:], ones[:], start=True, stop=True)
            fin = fpool.tile([1, 1], f32, tag="fin")
            nc.vector.scalar_tensor_tensor(
                fin[:], dacc[:], BETA / 2.0, psum[:], Alu.mult, Alu.add,
            )
            nc.sync.dma_start(out.ap(), fin[:])

    nc.compile()
    return nc


def _get_prog():
    global _PROG
    if _PROG is None:
        _PROG = _build()
    return _PROG


def kernel(logits, targets, weights, deltas):
    from concourse.bass_utils import run_bass_kernel_spmd

    nc = _get_prog()

    lg = np.ascontiguousarray(logits, dtype=np.float32)
    tf = np.ascontiguousarray(targets).astype(np.float32)
    wf = np.ascontiguousarray(weights, dtype=np.float32).reshape(-1)
    d0 = np.zeros(192, dtype=np.float32)
    d0[:189] = np.asarray(deltas, dtype=np.float32)[1:].reshape(-1)
    dz = np.zeros(192, dtype=np.float32)
    in_maps = []
    for c in range(NCORES):
        in_maps.append({
            "logits": lg[c * BC:(c + 1) * BC],
            "targets": tf[c * BC:(c + 1) * BC],
            "wts": wf[c * WPER:(c + 1) * WPER],
            "dls": d0 if c == 0 else dz,
        })

    res = run_bass_kernel_spmd(nc, in_maps, core_ids=list(range(NCORES)))
    total = sum(float(res.results[c]["out"][0, 0]) for c in range(NCORES))
    return np.array(total, dtype=np.float32)



# revision 3
# speedup vs baseline: 1.0012x; 1.0012x over previous
"""CoGOL ordinal-logistic loss on 8 Trainium2 NeuronCores.

Math (per sample, target t in [1,64], logits x[0..62], x_62 unused):
  masked-logsigmoid sum per row (see reference) equals -Q_i - ln2*[t>=2] with
    Q_i = sum_{j=0}^{t-3} x_j  +  sum_{j=0}^{61} sp(-x_j)  -  sp(-x_{t-2})*[2<=t<=63]
  (sp = softplus). Using sgm = sigmoid(x):  sp(-x_j) = -ln(sgm_j), so
    sum_j sp(-x_j) - sp(-x_{t-2}) = -ln( prod_{j != t-2} sgm_j ).
  result = [sum_i Q_i + ln2*count(t>=2)]/B + a/2*sum(w^2) + b/2*sum(d[1:]^2)

Device (per core, 65536 rows):
  - Act: sgm = sigmoid(x[:, :62]) -> bf16
  - DVE: oh = (iota(j+2) == t) bf16 one-hot [63 wide, col 62 <-> t=64]
         msel = max(sgm, oh[:62])   (replaces col t-2 with 1.0)
         half-row products of msel -> [rows, 2] bf16
  - PE : C += x_bf16^T @ oh accumulated in PSUM -> [62, 63] class-sum matrix
         (x_bf16 = free stride-2 bitcast view = truncated bf16 of f32 logits)
  - Act: Ln over collected products, accum -> per-partition sums
Host: prefix term = sum_{j<=c-1} C[j,c]; ln2 count; regularizers; final sum.
"""

import sys

sys.path.insert(0, "/opt/trn_rl_repo")

import numpy as np

ALPHA = 0.01
BETA = 0.05
B = 524288
KM1 = 63
NC62 = 62
NCORES = 8
BC = B // NCORES              # 65536 rows per core
R = 32                        # rows per partition per tile
NTILES = BC // (128 * R)      # 16
RTOT = BC // 128              # 512 rows per partition total
LN2 = 0.6931471805599453

_PROG = None


def _build():
    import concourse.bacc as bacc
    import concourse.tile as tile
    from concourse import mybir

    f32 = mybir.dt.float32
    bf16 = mybir.dt.bfloat16
    i32 = mybir.dt.int32
    Alu = mybir.AluOpType
    Act = mybir.ActivationFunctionType

    nc = bacc.Bacc("TRN2", target_bir_lowering=False, debug=False,
                   num_devices=NCORES)

    logits = nc.dram_tensor("logits", [BC, KM1], f32, kind="ExternalInput")
    targets = nc.dram_tensor("targets", [BC], bf16, kind="ExternalInput")
    out_ln = nc.dram_tensor("lnacc", [128, 1], f32, kind="ExternalOutput")
    out_c = nc.dram_tensor("cmat", [NC62, KM1], f32, kind="ExternalOutput")

    with tile.TileContext(nc) as tc:
        with (
            tc.tile_pool(name="const", bufs=1) as cpool,
            tc.tile_pool(name="x", bufs=3) as xpool,
            tc.tile_pool(name="t", bufs=3) as tpool,
            tc.tile_pool(name="s", bufs=3) as spool,
            tc.tile_pool(name="o", bufs=3) as opool,
            tc.tile_pool(name="fin", bufs=1) as fpool,
            tc.tile_pool(name="ps", bufs=1, space="PSUM") as ppool,
        ):
            # iota j+2 (j=0..62) as bf16, broadcast per row-block via 0-step AP
            iota_i = cpool.tile([128, KM1], i32)
            nc.gpsimd.iota(iota_i[:], pattern=[[1, KM1]], base=2,
                           channel_multiplier=0)
            iota_f = cpool.tile([128, KM1], f32)
            nc.vector.tensor_copy(iota_f[:], iota_i[:])
            iota_b = cpool.tile([128, KM1], bf16)
            nc.vector.tensor_copy(iota_b[:], iota_f[:])

            # collected half-row products, [128, RTOT, 2] bf16
            lncol = cpool.tile([128, RTOT, 2], bf16)
            psum_c = ppool.tile([NC62, KM1], f32)

            for k in range(NTILES):
                rows0 = k * R * 128
                roff = k * R
                xt = xpool.tile([128, R, KM1], f32, tag="x")
                nc.sync.dma_start(
                    xt[:],
                    logits.ap()[rows0:rows0 + R * 128, :]
                    .rearrange("(p r) c -> p r c", p=128),
                )
                tt = tpool.tile([128, R], bf16, tag="t")
                nc.sync.dma_start(
                    tt[:],
                    targets.ap()[rows0:rows0 + R * 128]
                    .rearrange("(p r) -> p r", p=128),
                )

                # sgm = sigmoid(x) -> bf16
                sgm = spool.tile([128, R, NC62], bf16, tag="sgm")
                nc.scalar.activation(sgm[:], xt[:, :, 0:NC62], Act.Sigmoid)

                # one-hot: oh[p, r, j] = (t == j+2), 63 wide
                oh = opool.tile([128, R, KM1], bf16, tag="oh")
                nc.vector.tensor_tensor(
                    oh[:], tt[:, :, None].to_broadcast([128, R, KM1]),
                    iota_b[:][:, None, :].to_broadcast([128, R, KM1]),
                    Alu.is_equal,
                )

                # msel = max(sgm, oh[:, :, :62]) in-place
                nc.vector.tensor_tensor(
                    sgm[:], sgm[:], oh[:, :, 0:NC62], Alu.max)

                # half-row products -> lncol[:, roff:roff+R, :]
                nc.vector.tensor_reduce(
                    lncol[:, roff:roff + R, :],
                    sgm[:].rearrange("p r (h k) -> p r h k", h=2),
                    axis=mybir.AxisListType.X,
                    op=Alu.mult,
                )

                # PSUM C += x_bf16^T @ oh, contracting the 128 partitions
                xv = xt[:].bitcast(bf16).rearrange(
                    "p r (c two) -> p r c two", two=2)
                for rr in range(R):
                    nc.tensor.matmul(
                        psum_c[:],
                        xv[:, rr, 0:NC62, 1],
                        oh[:, rr, :],
                        start=(k == 0 and rr == 0),
                        stop=(k == NTILES - 1 and rr == R - 1),
                    )

            # final: Ln over all collected products, accumulate per partition
            ln_out = fpool.tile([128, RTOT, 2], f32, tag="lnout")
            lnacc = fpool.tile([128, 1], f32, tag="lnacc")
            nc.scalar.activation(
                ln_out[:], lncol[:], Act.Ln, accum_out=lnacc[:])
            nc.sync.dma_start(out_ln.ap(), lnacc[:])

            cfin = fpool.tile([NC62, KM1], f32, tag="cfin")
            nc.scalar.copy(cfin[:], psum_c[:])
            nc.sync.dma_start(out_c.ap(), cfin[:])

    nc.compile()
    return nc


def _get_prog():
    global _PROG
    if _PROG is None:
        _PROG = _build()
    return _PROG


# host-side prefix weights: C[j, c] counts class t=c+2; row j contributes to
# the prefix sum iff j <= t-3 = c-1
_TRI = (np.arange(NC62)[:, None] <= np.arange(KM1)[None, :] - 1).astype(
    np.float64)


def _in_maps(logits, targets):
    import ml_dtypes

    lg = np.ascontiguousarray(logits, dtype=np.float32)
    tb = np.ascontiguousarray(targets).astype(np.float32).astype(
        ml_dtypes.bfloat16)
    return [
        {
            "logits": lg[c * BC:(c + 1) * BC],
            "targets": tb[c * BC:(c + 1) * BC],
        }
        for c in range(NCORES)
    ]


def kernel(logits, targets, weights, deltas):
    from concourse.bass_utils import run_bass_kernel_spmd

    nc = _get_prog()
    res = run_bass_kernel_spmd(nc, _in_maps(logits, targets),
                               core_ids=list(range(NCORES)))

    total = 0.0
    for c in range(NCORES):
        r = res.results[c]
        total += float(np.asarray(r["cmat"], np.float64).__mul__(_TRI).sum())
        total -= float(np.asarray(r["lnacc"], np.float64).sum())

    t64 = np.asarray(targets)
    total += LN2 * float(np.count_nonzero(t64 >= 2))

    w = np.asarray(weights, np.float64)
    d = np.asarray(deltas, np.float64)
    result = (total / B + ALPHA / 2.0 * np.sum(w * w)
              + BETA / 2.0 * np.sum(d[1:] * d[1:]))
    return np.array(result, dtype=np.float32)


# revision 6
# speedup vs baseline: 1.1370x; 1.1356x over previous
"""CoGOL ordinal-logistic loss on 8 Trainium2 NeuronCores.

Math (per sample, target t in [1,64], logits x[0..62], x_62 unused):
  masked-logsigmoid sum per row (see reference) equals -Q_i - ln2*[t>=2] with
    Q_i = sum_{j=0}^{t-3} x_j  +  sum_{j=0}^{61} sp(-x_j)  -  sp(-x_{t-2})*[2<=t<=63]
  (sp = softplus). Using sgm = sigmoid(x):  sp(-x_j) = -ln(sgm_j), so
    sum_j sp(-x_j) - sp(-x_{t-2}) = -ln( prod_{j != t-2} sgm_j ).
  result = [sum_i Q_i + ln2*count(t>=2)]/B + a/2*sum(w^2) + b/2*sum(d[1:]^2)

Layout trick: all elementwise tensors are stored TRANSPOSED per tile,
[128, col, row] instead of [128, row, col].  In that orientation the
per-row broadcast of t has its stride-0 dim OUTER and a stride-1 inner
dim, so every DVE op (is_equal one-hot, max-fold, product tree) keeps
the packed-bf16 2x mode (the row-major layout ran them at 1x).

Device (per core, 65536 rows):
  - Act   : sgm_t = sigmoid(x^T) -> bf16 [128, 64, r] (cols 62/63 = 1.0 pad;
            Act engine cost ignores strides so the transposed read is free)
  - DVE   : oh_t = (iota(c+2) == t) bf16 (2x), msel = max(sgm_t, oh_t) (2x)
  - mixed : per-row product via pairwise tree 64->32->16 (2x tensor_tensor,
            per-tile on DVE or gpsimd) or direct full-width Ln on Act --
            per-tile knob to balance engines
  - PE    : C += x_bf16^T @ oh accumulated in PSUM -> [62, 63] class sums
            (x_bf16 = free stride-2 bitcast view = truncated bf16 logits)
  - Act   : Ln over collected [128, 16, 512] partials, accum per partition
Host: prefix term = sum_{j<=c-1} C[j,c]; ln2 count; regularizers; final sum.
"""

import sys

sys.path.insert(0, "/opt/trn_rl_repo")

import numpy as np

ALPHA = 0.01
BETA = 0.05
B = 524288
KM1 = 63
NC62 = 62
NCORES = 8
BC = B // NCORES              # 65536 rows per core
RTOT = BC // 128              # 512 rows per partition total
R = 32                        # max rows per partition per tile
SIZES = [8, 8, 8, 8] + [32] * 15   # small warmup tiles, then full tiles
assert sum(SIZES) == RTOT
TREEW = 16                    # partial products kept per row by the tree
LN2 = 0.6931471805599453

# per-tile engine for the product stage: 'dve'/'gps' = pairwise tree on that
# engine (Ln reads the 16 partials later), 'act' = full-width Ln directly
TREE = ['dve'] * 19
assert len(TREE) == len(SIZES)
NACT = 2 + sum(1 for e in TREE if e == 'act')   # accum cols: final ln + acts

_PROG = None


def _build():
    import concourse.bacc as bacc
    import concourse.tile as tile
    from concourse import mybir

    f32 = mybir.dt.float32
    bf16 = mybir.dt.bfloat16
    i32 = mybir.dt.int32
    Alu = mybir.AluOpType
    Act = mybir.ActivationFunctionType

    nc = bacc.Bacc("TRN2", target_bir_lowering=False, debug=False,
                   num_devices=NCORES)

    logits = nc.dram_tensor("logits", [BC, KM1], f32, kind="ExternalInput")
    targets = nc.dram_tensor("targets", [BC], bf16, kind="ExternalInput")
    out_ln = nc.dram_tensor("lnacc", [128, NACT], f32, kind="ExternalOutput")
    out_c = nc.dram_tensor("cmat", [NC62, KM1], f32, kind="ExternalOutput")

    with tile.TileContext(nc) as tc:
        with (
            tc.tile_pool(name="const", bufs=1) as cpool,
            tc.tile_pool(name="x", bufs=3) as xpool,
            tc.tile_pool(name="t", bufs=3) as tpool,
            tc.tile_pool(name="s", bufs=3) as spool,
            tc.tile_pool(name="o", bufs=3) as opool,
            tc.tile_pool(name="lns", bufs=2) as lpool,
            tc.tile_pool(name="fin", bufs=1) as fpool,
            tc.tile_pool(name="ps", bufs=1, space="PSUM") as ppool,
        ):
            # iota_rep_t[p, c, r] = c + 2, materialized packed so the
            # transposed one-hot compare keeps all operands stride-1
            iota_i = cpool.tile([128, 64], i32)
            nc.gpsimd.iota(iota_i[:], pattern=[[1, 64]], base=2,
                           channel_multiplier=0)
            iota_f = cpool.tile([128, 64], f32)
            nc.vector.tensor_copy(iota_f[:], iota_i[:])
            iota_b = cpool.tile([128, 64], bf16)
            nc.vector.tensor_copy(iota_b[:], iota_f[:])
            iota_rep = cpool.tile([128, 64, R], bf16)
            nc.vector.tensor_copy(
                iota_rep[:], iota_b[:, :, None].to_broadcast([128, 64, R]))

            # collected partial products, [128, TREEW, RTOT] bf16
            lncol = cpool.tile([128, TREEW, RTOT], bf16)
            acc = cpool.tile([128, NACT], f32)
            psum_c = ppool.tile([NC62, KM1], f32)
            nacti = 2

            roff = 0
            for k, r in enumerate(SIZES):
                rows0 = roff * 128
                xt = xpool.tile([128, R, KM1], f32, tag="x")
                nc.sync.dma_start(
                    xt[:, :r, :],
                    logits.ap()[rows0:rows0 + r * 128, :]
                    .rearrange("(p r) c -> p r c", p=128),
                )
                tt = tpool.tile([128, R], bf16, tag="t")
                nc.sync.dma_start(
                    tt[:, :r],
                    targets.ap()[rows0:rows0 + r * 128]
                    .rearrange("(p r) -> p r", p=128),
                )

                # sgm_t[p, c, r] = sigmoid(x[p, r, c]) -> bf16, 1.0 pad c=62/63
                sgm = spool.tile([128, 64, R], bf16, tag="sgm")
                nc.vector.memset(sgm[:, NC62:64, :r], 1.0)
                nc.scalar.activation(
                    sgm[:, 0:NC62, :r],
                    xt[:, :r, 0:NC62].rearrange("p r c -> p c r"),
                    Act.Sigmoid)

                # one-hot: oh[p, c, r] = (t[p, r] == c+2), 63 cols used
                oh = opool.tile([128, 64, R], bf16, tag="oh")
                nc.vector.tensor_tensor(
                    oh[:, 0:KM1, :r],
                    tt[:, None, :r].to_broadcast([128, KM1, r]),
                    iota_rep[:, 0:KM1, :r],
                    Alu.is_equal,
                )

                # msel = max(sgm, oh) in-place on the 62 real columns (2x)
                nc.vector.tensor_tensor(
                    sgm[:, 0:NC62, :r], sgm[:, 0:NC62, :r],
                    oh[:, 0:NC62, :r], Alu.max)

                # per-row product of msel
                eng = TREE[k]
                if eng == 'act':
                    lnsc = lpool.tile([128, NC62, R], f32, tag="lnsc")
                    nc.scalar.activation(
                        lnsc[:, :, :r], sgm[:, 0:NC62, :r], Act.Ln,
                        accum_out=acc[:, nacti:nacti + 1])
                    nacti += 1
                else:
                    e = nc.vector if eng == 'dve' else nc.gpsimd
                    e.tensor_tensor(
                        sgm[:, 0:32, :r], sgm[:, 0:32, :r], sgm[:, 32:64, :r],
                        Alu.mult)
                    e.tensor_tensor(
                        lncol[:, :, roff:roff + r], sgm[:, 0:TREEW, :r],
                        sgm[:, TREEW:32, :r], Alu.mult)

                # PSUM C += x_bf16^T @ oh, contracting the 128 partitions
                xv = xt[:].bitcast(bf16).rearrange(
                    "p r (c two) -> p r c two", two=2)
                for rr in range(r):
                    nc.tensor.matmul(
                        psum_c[:],
                        xv[:, rr, 0:NC62, 1],
                        oh[:, 0:KM1, rr],
                        start=(k == 0 and rr == 0),
                        stop=(k == len(SIZES) - 1 and rr == r - 1),
                    )
                roff += r

            # final: Ln over collected tree partials, accumulate per partition
            ln_out = fpool.tile([128, TREEW, RTOT], f32, tag="lnout")
            nc.scalar.activation(
                ln_out[:], lncol[:], Act.Ln, accum_out=acc[:, 0:1])
            nc.vector.memset(acc[:, 1:2], 0.0)
            nc.sync.dma_start(out_ln.ap(), acc[:])

            cfin = fpool.tile([NC62, KM1], f32, tag="cfin")
            nc.scalar.copy(cfin[:], psum_c[:])
            nc.sync.dma_start(out_c.ap(), cfin[:])

    nc.compile()
    return nc


def _get_prog():
    global _PROG
    if _PROG is None:
        _PROG = _build()
    return _PROG


# host-side prefix weights: C[j, c] counts class t=c+2; row j contributes to
# the prefix sum iff j <= t-3 = c-1
_TRI = (np.arange(NC62)[:, None] <= np.arange(KM1)[None, :] - 1).astype(
    np.float64)


def _in_maps(logits, targets):
    import ml_dtypes

    lg = np.ascontiguousarray(logits, dtype=np.float32)
    tb = np.ascontiguousarray(targets).astype(np.float32).astype(
        ml_dtypes.bfloat16)
    return [
        {
            "logits": lg[c * BC:(c + 1) * BC],
            "targets": tb[c * BC:(c + 1) * BC],
        }
        for c in range(NCORES)
    ]


def kernel(logits, targets, weights, deltas):
    from concourse.bass_utils import run_bass_kernel_spmd

    nc = _get_prog()
    res = run_bass_kernel_spmd(nc, _in_maps(logits, targets),
                               core_ids=list(range(NCORES)))

    total = 0.0
    for c in range(NCORES):
        r = res.results[c]
        total += float((np.asarray(r["cmat"], np.float64) * _TRI).sum())
        total -= float(np.asarray(r["lnacc"], np.float64).sum())

    t64 = np.asarray(targets)
    total += LN2 * float(np.count_nonzero(t64 >= 2))

    w = np.asarray(weights, np.float64)
    d = np.asarray(deltas, np.float64)
    result = (total / B + ALPHA / 2.0 * np.sum(w * w)
              + BETA / 2.0 * np.sum(d[1:] * d[1:]))
    return np.array(result, dtype=np.float32)
